# revision 1
# baseline (speedup 1.0000x reference)
"""GraphSAGE 2-layer encoder on 8 TRN2 NeuronCores.

Strategy (dst-sharded, "transposed world"):
- Nodes sharded 8x12500 by dst range; core k computes output rows for its nodes.
- Per layer, per core: edge messages x[src] arrive as a slot array (128-slot
  tiles, dst-sorted, grouped into 128-node cells); segment-sum runs on the PE
  as one-hot matmuls (S built on-device by iota-compare on DVE), accumulating
  feature-major aggregates [128f, nodes] in pre-zeroed PSUM banks; DVE scales
  by 1/deg; stationary W_l/W_r matmuls transform (rhs = mean^T, h_own^T);
  ScalarE fuses bias+ReLU; the transposed output shard [128, N_CANON] stores
  to DRAM. Two launches of one compiled program (layer 1, layer 2); the host
  reassembles h1 between launches and expands next-layer messages.
"""
import os
import numpy as np
import ml_dtypes

import concourse.bass as bass
import concourse.tile as tile
from concourse import bacc, mybir
from concourse.bass_utils import run_bass_kernel_spmd

N_NODES = 100000
N_CORES = 8
OWN = N_NODES // N_CORES          # 12500
D = 128
CELL = 128                        # node-columns per cell (= S width = MM N)
N_CELLS = (OWN + CELL - 1) // CELL  # 98
N_CANON = N_CELLS * CELL          # 12544
BANK_CELLS = 4                    # cells per PSUM bank (512 cols)
N_BANKS = (N_CELLS + BANK_CELLS - 1) // BANK_CELLS  # 25

BF16 = mybir.dt.bfloat16
F32 = mybir.dt.float32
F32R = mybir.dt.float32r

_cache = {}


def _build_program(T_cells):
    """One layer's SPMD program. T_cells[c] = #128-slot tiles for cell c."""
    TOT_T = int(np.sum(T_cells))
    nc = bacc.Bacc()

    msgs_d = nc.declare_dram_parameter("msgs", [128, TOT_T * D], BF16, isOutput=False)
    dstc_d = nc.declare_dram_parameter("dstc", [128, max(TOT_T, 1)], BF16, isOutput=False)
    inv_d = nc.declare_dram_parameter("invc", [1, N_CANON], F32, isOutput=False)
    ht_d = nc.declare_dram_parameter("ht", [128, N_CANON], F32R, isOutput=False)
    wl_d = nc.declare_dram_parameter("wl", [128, 128], F32R, isOutput=False)
    wr_d = nc.declare_dram_parameter("wr", [128, 128], F32R, isOutput=False)
    b_d = nc.declare_dram_parameter("bias", [128, 1], F32, isOutput=False)
    iota_d = nc.declare_dram_parameter("iota", [1, CELL], BF16, isOutput=False)
    out_d = nc.declare_dram_parameter("outT", [128, N_CANON], F32, isOutput=True)

    # bank plan: (cell_start, n_cells, tile ranges)
    banks = []
    t0 = 0
    for bk in range(N_BANKS):
        c0 = bk * BANK_CELLS
        ncell = min(BANK_CELLS, N_CELLS - c0)
        tiles = []  # (t_global, cell_off_in_bank)
        for ci in range(ncell):
            for _ in range(T_cells[c0 + ci]):
                tiles.append((t0, ci))
                t0 += 1
        banks.append((c0, ncell, tiles))

    T_BANK_MAX = max(max(len(b[2]) for b in banks), 1)

    with tile.TileContext(nc) as tc:
        with (
            tc.tile_pool(name="singles", bufs=1) as singles,
            tc.tile_pool(name="msgp", bufs=3) as msgp,
            tc.tile_pool(name="sp", bufs=3) as sp,
            tc.tile_pool(name="htp", bufs=2) as htp,
            tc.tile_pool(name="mp", bufs=2) as mp,
            tc.tile_pool(name="outp", bufs=3) as outp,
            tc.tile_pool(name="psa", bufs=3, space="PSUM") as psa,
            tc.tile_pool(name="pst", bufs=2, space="PSUM") as pst,
        ):
            # ---- constants ----
            dstc_t = singles.tile([128, max(TOT_T, 1)], BF16)
            nc.sync.dma_start(out=dstc_t[:], in_=dstc_d[:])
            iota_t = singles.tile([128, CELL], BF16)
            nc.gpsimd.dma_start(
                out=iota_t[:],
                in_=bass.AP(tensor=iota_d[:].tensor, offset=0, ap=[[0, 128], [1, CELL]]),
            )
            inv_t = singles.tile([128, N_CANON], F32)
            nc.gpsimd.dma_start(
                out=inv_t[:],
                in_=bass.AP(tensor=inv_d[:].tensor, offset=0, ap=[[0, 128], [1, N_CANON]]),
            )
            wl_t = singles.tile([128, 128], F32R)
            nc.sync.dma_start(out=wl_t[:], in_=wl_d[:])
            wr_t = singles.tile([128, 128], F32R)
            nc.sync.dma_start(out=wr_t[:], in_=wr_d[:])
            b_t = singles.tile([128, 1], F32)
            nc.sync.dma_start(out=b_t[:], in_=b_d[:])
            zeros_t = singles.tile([128, 512], BF16)
            nc.vector.memset(zeros_t[:], 0.0)

            # ---- per-bank pipeline ----
            for bk, (c0, ncell, tiles) in enumerate(banks):
                bankcols = ncell * CELL
                nt = len(tiles)
                psum_agg = psa.tile([128, bankcols], F32)
                # clear bank (sets has_written)
                nc.tensor.matmul(
                    psum_agg[:], zeros_t[:, :128], zeros_t[:, :bankcols],
                    start=True, stop=(nt == 0),
                )
                if nt:
                    tg0 = tiles[0][0]
                    msg_t = msgp.tile([128, T_BANK_MAX, D], BF16)
                    nc.sync.dma_start(
                        out=msg_t[:, :nt, :],
                        in_=msgs_d[:, tg0 * D : (tg0 + nt) * D].rearrange(
                            "p (t d) -> p t d", d=D
                        ),
                    )
                    s_t = sp.tile([128, T_BANK_MAX, CELL], BF16)
                    dap = dstc_t[:, tg0 : tg0 + nt].to_broadcast([128, nt, CELL])
                    iap = bass.AP(
                        tensor=iota_t[:].tensor, offset=iota_t[:].offset,
                        ap=[iota_t[:].ap[0], [0, nt], [1, CELL]],
                    )
                    nc.vector.tensor_tensor(
                        out=s_t[:, :nt, :], in0=dap, in1=iap,
                        op=mybir.AluOpType.is_equal,
                    )
                    for i, (tg, ci) in enumerate(tiles):
                        nc.tensor.matmul(
                            psum_agg[:, ci * CELL : (ci + 1) * CELL],
                            msg_t[:, i, :],
                            s_t[:, i, :],
                            start=False,
                            stop=(i == nt - 1),
                        )
                # mean^T = psum * inv_cnt
                mean_t = mp.tile([128, bankcols], F32R)
                nc.vector.tensor_tensor(
                    out=mean_t[:], in0=psum_agg[:],
                    in1=inv_t[:, c0 * CELL : c0 * CELL + bankcols],
                    op=mybir.AluOpType.mult,
                )
                # transform: out^T = W_l^T mean^T + W_r^T h_own^T
                ht_t = htp.tile([128, bankcols], F32R)
                nc.sync.dma_start(
                    out=ht_t[:], in_=ht_d[:, c0 * CELL : c0 * CELL + bankcols]
                )
                psum_o = pst.tile([128, bankcols], F32)
                nc.tensor.matmul(
                    psum_o[:], wl_t[:], mean_t[:],
                    start=True, stop=False,
                )
                nc.tensor.matmul(
                    psum_o[:], wr_t[:], ht_t[:],
                    start=False, stop=True,
                )
                out_t = outp.tile([128, bankcols], F32)
                nc.scalar.activation(
                    out=out_t[:], in_=psum_o[:],
                    func=mybir.ActivationFunctionType.Relu,
                    bias=b_t[:], scale=1.0,
                )
                nc.sync.dma_start(
                    out=out_d[:, c0 * CELL : c0 * CELL + bankcols], in_=out_t[:]
                )
    nc.finalize()
    return nc, TOT_T


def _schedule(edge_index):
    """Per-core slot schedule shared by both layers."""
    src = np.asarray(edge_index[0], dtype=np.int64)
    dst = np.asarray(edge_index[1], dtype=np.int64)
    deg = np.bincount(dst, minlength=N_NODES).astype(np.float32)
    inv_full = 1.0 / np.maximum(deg, 1.0)

    cores = []
    cell_counts = np.zeros((N_CORES, N_CELLS), np.int64)
    for k in range(N_CORES):
        m = (dst // OWN) == k
        s_k = src[m]
        dloc = dst[m] - k * OWN
        order = np.argsort(dloc, kind="stable")
        s_k, dloc = s_k[order], dloc[order]
        cell = dloc // CELL
        cell_counts[k] = np.bincount(cell, minlength=N_CELLS)
        cores.append((s_k, dloc, cell))

    T_cells = np.ceil(cell_counts.max(axis=0) / 128.0).astype(np.int64)
    TOT_T = int(T_cells.sum())
    TOT_S = TOT_T * 128
    tile_base = np.concatenate([[0], np.cumsum(T_cells)])[:-1]  # first tile of cell
    slot_base = tile_base * 128

    sched = []
    for k in range(N_CORES):
        s_k, dloc, cell = cores[k]
        n = len(s_k)
        cnt = cell_counts[k]
        cstart = np.concatenate([[0], np.cumsum(cnt)])[:-1]
        rank = np.arange(n) - cstart[cell]
        slot = slot_base[cell] + rank
        slot_src = np.zeros(TOT_S, np.int64)
        slot_src[slot] = s_k
        dstc_flat = np.full(TOT_S, -1.0, np.float32)
        dstc_flat[slot] = (dloc % CELL).astype(np.float32)
        # slot s -> (t = s//128, p = s%128); device reads dstc as [p, t]
        dstc_arr = dstc_flat.reshape(TOT_T, 128).T.astype(ml_dtypes.bfloat16)
        inv_row = np.ones((1, N_CANON), np.float32)
        inv_row[0, :OWN] = inv_full[k * OWN : (k + 1) * OWN]
        sched.append((slot_src, np.ascontiguousarray(dstc_arr), inv_row))
    return sched, T_cells, TOT_T, TOT_S


def _layer_inputs(sched, TOT_T, TOT_S, h, W_l, b_l, W_r):
    """Build per-core in_maps for one layer."""
    h_bf = h.astype(ml_dtypes.bfloat16)
    iota = np.arange(CELL).astype(ml_dtypes.bfloat16).reshape(1, CELL)
    in_maps = []
    for k in range(N_CORES):
        slot_src, dstc_arr, inv_row = sched[k]
        g = h_bf[slot_src]  # [TOT_S, 128]
        msgs = np.ascontiguousarray(
            g.reshape(TOT_T, 128, D).transpose(1, 0, 2).reshape(128, TOT_T * D)
        )
        ht = np.zeros((128, N_CANON), np.float32)
        ht[:, :OWN] = h[k * OWN : (k + 1) * OWN].T
        in_maps.append({
            "msgs": msgs,
            "dstc": dstc_arr,
            "invc": inv_row,
            "ht": ht,
            "wl": np.ascontiguousarray(W_l.astype(np.float32)),
            "wr": np.ascontiguousarray(W_r.astype(np.float32)),
            "bias": np.ascontiguousarray(b_l.astype(np.float32).reshape(128, 1)),
            "iota": iota,
        })
    return in_maps


def _run_layer(nc, in_maps, trace):
    import time as _time
    t0 = _time.perf_counter()
    res = run_bass_kernel_spmd(
        nc, in_maps, core_ids=list(range(N_CORES)), trace=False
    )
    kernel.last_launch_wall_ns = int((_time.perf_counter() - t0) * 1e9)
    h = np.empty((N_NODES, D), np.float32)
    for k in range(N_CORES):
        h[k * OWN : (k + 1) * OWN] = np.asarray(res.results[k]["outT"])[:, :OWN].T
    t = res.exec_time_ns
    return h, (int(t) if t is not None else None)


def kernel(x, edge_index, W_l0, b_l0, W_r0, W_l1, b_l1, W_r1):
    x = np.asarray(x, dtype=np.float32)
    trace = os.environ.get("KERNEL_TRACE", "0") == "1"

    key = "prog"
    sched, T_cells, TOT_T, TOT_S = _schedule(edge_index)
    tkey = (key, tuple(T_cells.tolist()))
    if tkey not in _cache:
        _cache[tkey] = _build_program(T_cells)
    nc, _ = _cache[tkey]

    h1, t1 = _run_layer(nc, _layer_inputs(sched, TOT_T, TOT_S, x, W_l0, b_l0, W_r0), trace)
    w1 = kernel.last_launch_wall_ns
    h2, t2 = _run_layer(nc, _layer_inputs(sched, TOT_T, TOT_S, h1, W_l1, b_l1, W_r1), trace)
    w2 = kernel.last_launch_wall_ns
    if t1 is not None and t2 is not None:
        kernel.last_exec_ns = t1 + t2
    else:
        # NTFF profiling hook unavailable under this axon client; report
        # 2x the warm launch wall (incl. host<->device transfer) as an
        # upper bound (first launch wall also includes NEFF compile).
        kernel.last_exec_ns = 2 * min(w1, w2)
    return h2



# revision 6
# speedup vs baseline: 8.3299x; 8.3299x over previous
"""GraphSAGE 2-layer encoder on 8 TRN2 NeuronCores — single-launch design.

Strategy (dst-sharded, on-device gather, one launch for both layers):
- Nodes sharded 8x12500 by dst range; core k computes output rows for its
  nodes. x ships bf16-sharded (3.2MB/core); an on-device AllGather builds the
  full node table in DRAM.
- Per layer, per core: per-edge messages are gathered ON DEVICE from the full
  table via indirect DMA (128 rows per instruction, one per 128-slot tile);
  segment-sum runs on the PE as one-hot matmuls (S built by iota-compare on
  DVE), accumulating feature-major aggregates in PSUM banks; DVE scales by
  1/deg; stationary W_l/W_r matmuls transform; the self term h_own^T comes
  from PE-transposing own rows loaded from the local shard; ScalarE fuses
  bias+ReLU.
- Layer 1's output is PE-transposed back to node-major bf16, stored to the
  local shard, and AllGathered into a second full table for layer 2's
  gathers. Layer 2 writes the transposed output shard [128, N_CANON] bf16.
- One compiled program, one launch: host only shards inputs and reassembles
  the output.
"""
import os
import numpy as np
import ml_dtypes

import concourse.bass as bass
import concourse.tile as tile
from concourse import bacc, mybir
from concourse.bass_utils import run_bass_kernel_spmd
from concourse.masks import make_identity

N_NODES = 100000
N_CORES = 8
OWN = N_NODES // N_CORES          # 12500
D = 128
CELL = 128                        # node-columns per cell (= S width = MM N)
N_CELLS = (OWN + CELL - 1) // CELL  # 98
N_CANON = N_CELLS * CELL          # 12544
BANK_CELLS = 4                    # cells per PSUM bank (512 cols)
N_BANKS = (N_CELLS + BANK_CELLS - 1) // BANK_CELLS  # 25
FULL_CELLS = OWN // CELL          # 97 full cells
TAIL_ROWS = OWN - FULL_CELLS * CELL  # 84 rows in the last (partial) cell

BF16 = mybir.dt.bfloat16
F32 = mybir.dt.float32
F32R = mybir.dt.float32r
I32 = mybir.dt.int32

_cache = {}


def _build_program(T_cells):
    """Both layers' SPMD program. T_cells[c] = #128-slot tiles for cell c."""
    TOT_T = int(np.sum(T_cells))
    nc = bacc.Bacc(num_devices=N_CORES)

    xsh_d = nc.declare_dram_parameter("xsh", [OWN, D], BF16, isOutput=False)
    idx_d = nc.declare_dram_parameter("idx", [128, max(TOT_T, 1)], I32, isOutput=False)
    dstc_d = nc.declare_dram_parameter("dstc", [128, max(TOT_T, 1)], BF16, isOutput=False)
    inv_d = nc.declare_dram_parameter("invc", [1, N_CANON], F32, isOutput=False)
    wl0_d = nc.declare_dram_parameter("wl0", [128, 128], F32R, isOutput=False)
    wr0_d = nc.declare_dram_parameter("wr0", [128, 128], F32R, isOutput=False)
    wl1_d = nc.declare_dram_parameter("wl1", [128, 128], F32R, isOutput=False)
    wr1_d = nc.declare_dram_parameter("wr1", [128, 128], F32R, isOutput=False)
    b0_d = nc.declare_dram_parameter("b0", [128, 1], F32, isOutput=False)
    b1_d = nc.declare_dram_parameter("b1", [128, 1], F32, isOutput=False)
    iota_d = nc.declare_dram_parameter("iota", [1, CELL], BF16, isOutput=False)
    out_d = nc.declare_dram_parameter("outT", [128, N_CANON], BF16, isOutput=True)

    xb = nc.dram_tensor("xb", [OWN, D], BF16, kind="Internal")
    h_full = nc.dram_tensor("hfull", [N_NODES, D], BF16, kind="Internal")
    h1_sh = nc.dram_tensor("h1sh", [OWN, D], BF16, kind="Internal")
    h1_full = nc.dram_tensor("h1full", [N_NODES, D], BF16, kind="Internal")

    # bank plan: (cell_start, n_cells, tiles=[(t_global, cell_off_in_bank)])
    banks = []
    t0 = 0
    for bk in range(N_BANKS):
        c0 = bk * BANK_CELLS
        ncell = min(BANK_CELLS, N_CELLS - c0)
        tiles = []
        for ci in range(ncell):
            for _ in range(T_cells[c0 + ci]):
                tiles.append((t0, ci))
                t0 += 1
        banks.append((c0, ncell, tiles))
    T_BANK_MAX = max(max(len(b[2]) for b in banks), 1)

    with tile.TileContext(nc) as tc:
        with (
            tc.tile_pool(name="singles", bufs=1) as singles,
            tc.tile_pool(name="msgp", bufs=3) as msgp,
            tc.tile_pool(name="sp", bufs=3) as sp,
            tc.tile_pool(name="cellp", bufs=3) as cellp,
            tc.tile_pool(name="htp", bufs=2) as htp,
            tc.tile_pool(name="mp", bufs=2) as mp,
            tc.tile_pool(name="outp", bufs=3) as outp,
            tc.tile_pool(name="h1cp", bufs=3) as h1cp,
            tc.tile_pool(name="psa", bufs=3, space="PSUM") as psa,
            tc.tile_pool(name="pst", bufs=2, space="PSUM") as pst,
            tc.tile_pool(name="ptr", bufs=2, space="PSUM") as ptr,
        ):
            # ---- constants ----
            idx_t = singles.tile([128, max(TOT_T, 1)], I32)
            nc.sync.dma_start(out=idx_t[:], in_=idx_d[:])
            dstc_t = singles.tile([128, max(TOT_T, 1)], BF16)
            nc.sync.dma_start(out=dstc_t[:], in_=dstc_d[:])
            iota_t = singles.tile([128, CELL], BF16)
            nc.gpsimd.dma_start(
                out=iota_t[:],
                in_=bass.AP(tensor=iota_d[:].tensor, offset=0, ap=[[0, 128], [1, CELL]]),
            )
            inv_t = singles.tile([128, N_CANON], F32)
            nc.gpsimd.dma_start(
                out=inv_t[:],
                in_=bass.AP(tensor=inv_d[:].tensor, offset=0, ap=[[0, 128], [1, N_CANON]]),
            )
            wl0_t = singles.tile([128, 128], F32R)
            nc.sync.dma_start(out=wl0_t[:], in_=wl0_d[:])
            wr0_t = singles.tile([128, 128], F32R)
            nc.sync.dma_start(out=wr0_t[:], in_=wr0_d[:])
            wl1_t = singles.tile([128, 128], F32R)
            nc.sync.dma_start(out=wl1_t[:], in_=wl1_d[:])
            wr1_t = singles.tile([128, 128], F32R)
            nc.sync.dma_start(out=wr1_t[:], in_=wr1_d[:])
            b0_t = singles.tile([128, 1], F32)
            nc.sync.dma_start(out=b0_t[:], in_=b0_d[:])
            b1_t = singles.tile([128, 1], F32)
            nc.sync.dma_start(out=b1_t[:], in_=b1_d[:])
            zeros_t = singles.tile([128, 512], BF16)
            nc.vector.memset(zeros_t[:], 0.0)
            ident_b = singles.tile([128, 128], BF16)
            make_identity(nc, ident_b[:])
            ident_f = singles.tile([128, 128], F32)
            make_identity(nc, ident_f[:])

            # ---- AllGather x ----
            nc.gpsimd.dma_start(out=xb[:], in_=xsh_d[:])
            nc.gpsimd.collective_compute(
                "AllGather", mybir.AluOpType.bypass,
                replica_groups=[list(range(N_CORES))],
                ins=[xb[:]], outs=[h_full[:]],
            )

            def layer(src_full, self_src, wl_t, wr_t, b_t, is_last):
                for bk, (c0, ncell, tiles) in enumerate(banks):
                    bankcols = ncell * CELL
                    nt = len(tiles)
                    psum_agg = psa.tile([128, bankcols], F32)
                    nc.tensor.matmul(
                        psum_agg[:], zeros_t[:, :128], zeros_t[:, :bankcols],
                        start=True, stop=(nt == 0),
                    )
                    if nt:
                        tg0 = tiles[0][0]
                        msg_t = msgp.tile([128, T_BANK_MAX, D], BF16)
                        for i in range(nt):
                            nc.gpsimd.indirect_dma_start(
                                out=msg_t[:, i, :],
                                out_offset=None,
                                in_=src_full[:],
                                in_offset=bass.IndirectOffsetOnAxis(
                                    ap=idx_t[:, tg0 + i : tg0 + i + 1], axis=0
                                ),
                            )
                        s_t = sp.tile([128, T_BANK_MAX, CELL], BF16)
                        dap = dstc_t[:, tg0 : tg0 + nt].to_broadcast([128, nt, CELL])
                        iap = bass.AP(
                            tensor=iota_t[:].tensor, offset=iota_t[:].offset,
                            ap=[iota_t[:].ap[0], [0, nt], [1, CELL]],
                        )
                        nc.vector.tensor_tensor(
                            out=s_t[:, :nt, :], in0=dap, in1=iap,
                            op=mybir.AluOpType.is_equal,
                        )
                        for i, (tg, ci) in enumerate(tiles):
                            nc.tensor.matmul(
                                psum_agg[:, ci * CELL : (ci + 1) * CELL],
                                msg_t[:, i, :],
                                s_t[:, i, :],
                                start=False,
                                stop=(i == nt - 1),
                            )
                    # mean^T = psum * inv_cnt
                    mean_t = mp.tile([128, bankcols], F32R)
                    nc.vector.tensor_tensor(
                        out=mean_t[:], in0=psum_agg[:],
                        in1=inv_t[:, c0 * CELL : c0 * CELL + bankcols],
                        op=mybir.AluOpType.mult,
                    )
                    # self term: h_own^T per cell via PE transpose
                    ht_t = htp.tile([128, bankcols], F32R)
                    for ci in range(ncell):
                        cell = c0 + ci
                        r0 = cell * CELL
                        rows = min(CELL, OWN - r0)
                        if rows <= 0:
                            continue
                        cell_t = cellp.tile([128, 128], BF16)
                        nc.sync.dma_start(
                            out=cell_t[:rows, :], in_=self_src[r0 : r0 + rows, :]
                        )
                        psum_tr = ptr.tile([128, 128], BF16)
                        nc.tensor.transpose(psum_tr[:], cell_t[:], ident_b[:])
                        nc.vector.tensor_copy(
                            out=ht_t[:, ci * CELL : (ci + 1) * CELL], in_=psum_tr[:]
                        )
                    # transform: out^T = W_l^T mean^T + W_r^T h_own^T
                    psum_o = pst.tile([128, bankcols], F32)
                    nc.tensor.matmul(psum_o[:], wl_t[:], mean_t[:], start=True, stop=False)
                    nc.tensor.matmul(psum_o[:], wr_t[:], ht_t[:], start=False, stop=True)
                    out_t = outp.tile([128, bankcols], F32)
                    nc.scalar.activation(
                        out=out_t[:], in_=psum_o[:],
                        func=mybir.ActivationFunctionType.Relu,
                        bias=b_t[:], scale=1.0,
                    )
                    if is_last:
                        ob = h1cp.tile([128, bankcols], BF16)
                        nc.vector.tensor_copy(out=ob[:], in_=out_t[:])
                        nc.sync.dma_start(
                            out=out_d[:, c0 * CELL : c0 * CELL + bankcols], in_=ob[:]
                        )
                    else:
                        # node-major bf16 rows into h1_sh for next layer
                        for ci in range(ncell):
                            cell = c0 + ci
                            r0 = cell * CELL
                            rows = min(CELL, OWN - r0)
                            if rows <= 0:
                                continue
                            psum_tr = ptr.tile([128, 128], F32)
                            nc.tensor.transpose(
                                psum_tr[:],
                                out_t[:, ci * CELL : (ci + 1) * CELL],
                                ident_f[:],
                            )
                            h1c = h1cp.tile([128, 128], BF16)
                            nc.vector.tensor_copy(out=h1c[:], in_=psum_tr[:])
                            nc.sync.dma_start(
                                out=h1_sh[r0 : r0 + rows, :], in_=h1c[:rows, :]
                            )

            layer(h_full, xsh_d, wl0_t, wr0_t, b0_t, is_last=False)
            nc.gpsimd.collective_compute(
                "AllGather", mybir.AluOpType.bypass,
                replica_groups=[list(range(N_CORES))],
                ins=[h1_sh[:]], outs=[h1_full[:]],
            )
            layer(h1_full, h1_sh, wl1_t, wr1_t, b1_t, is_last=True)
    nc.finalize()
    return nc


def _schedule(edge_index):
    """Per-core slot schedule shared by both layers."""
    src = np.asarray(edge_index[0], dtype=np.int64)
    dst = np.asarray(edge_index[1], dtype=np.int64)
    deg = np.bincount(dst, minlength=N_NODES).astype(np.float32)
    inv_full = 1.0 / np.maximum(deg, 1.0)

    cores = []
    cell_counts = np.zeros((N_CORES, N_CELLS), np.int64)
    for k in range(N_CORES):
        m = (dst // OWN) == k
        s_k = src[m]
        dloc = dst[m] - k * OWN
        order = np.argsort(dloc, kind="stable")
        s_k, dloc = s_k[order], dloc[order]
        cell = dloc // CELL
        cell_counts[k] = np.bincount(cell, minlength=N_CELLS)
        cores.append((s_k, dloc, cell))

    T_cells = np.ceil(cell_counts.max(axis=0) / 128.0).astype(np.int64)
    TOT_T = int(T_cells.sum())
    TOT_S = TOT_T * 128
    tile_base = np.concatenate([[0], np.cumsum(T_cells)])[:-1]
    slot_base = tile_base * 128

    sched = []
    for k in range(N_CORES):
        s_k, dloc, cell = cores[k]
        n = len(s_k)
        cnt = cell_counts[k]
        cstart = np.concatenate([[0], np.cumsum(cnt)])[:-1]
        rank = np.arange(n) - cstart[cell]
        slot = slot_base[cell] + rank
        slot_src = np.zeros(TOT_S, np.int64)
        slot_src[slot] = s_k
        dstc_flat = np.full(TOT_S, -1.0, np.float32)
        dstc_flat[slot] = (dloc % CELL).astype(np.float32)
        # slot s -> (t = s//128, p = s%128); device reads [p, t]
        dstc_arr = dstc_flat.reshape(TOT_T, 128).T.astype(ml_dtypes.bfloat16)
        idx_arr = slot_src.reshape(TOT_T, 128).T.astype(np.int32)
        inv_row = np.ones((1, N_CANON), np.float32)
        inv_row[0, :OWN] = inv_full[k * OWN : (k + 1) * OWN]
        sched.append((
            np.ascontiguousarray(idx_arr),
            np.ascontiguousarray(dstc_arr),
            inv_row,
        ))
    return sched, T_cells, TOT_T


def _in_maps(sched, x, W_l0, b_l0, W_r0, W_l1, b_l1, W_r1):
    x_bf = x.astype(ml_dtypes.bfloat16)
    iota = np.arange(CELL).astype(ml_dtypes.bfloat16).reshape(1, CELL)
    f32 = lambda a: np.ascontiguousarray(a.astype(np.float32))
    maps = []
    for k in range(N_CORES):
        idx_arr, dstc_arr, inv_row = sched[k]
        maps.append({
            "xsh": np.ascontiguousarray(x_bf[k * OWN : (k + 1) * OWN]),
            "idx": idx_arr,
            "dstc": dstc_arr,
            "invc": inv_row,
            "wl0": f32(W_l0), "wr0": f32(W_r0),
            "wl1": f32(W_l1), "wr1": f32(W_r1),
            "b0": f32(b_l0).reshape(128, 1), "b1": f32(b_l1).reshape(128, 1),
            "iota": iota,
        })
    return maps


def kernel(x, edge_index, W_l0, b_l0, W_r0, W_l1, b_l1, W_r1):
    import time as _time
    x = np.asarray(x, dtype=np.float32)

    sched, T_cells, TOT_T = _schedule(edge_index)
    tkey = tuple(T_cells.tolist())
    if tkey not in _cache:
        _cache[tkey] = _build_program(T_cells)
    nc = _cache[tkey]

    maps = _in_maps(sched, x, W_l0, b_l0, W_r0, W_l1, b_l1, W_r1)

    def launch():
        t0 = _time.perf_counter()
        res = run_bass_kernel_spmd(nc, maps, core_ids=list(range(N_CORES)), trace=False)
        wall = int((_time.perf_counter() - t0) * 1e9)
        return res, wall

    res, w1 = launch()          # includes NEFF compile on first call
    res, w2 = launch()          # warm launch: transfer + execute + fetch
    kernel.last_exec_ns = min(w1, w2)

    h2 = np.empty((N_NODES, D), np.float32)
    for k in range(N_CORES):
        outT = np.asarray(res.results[k]["outT"]).astype(np.float32)
        h2[k * OWN : (k + 1) * OWN] = outT[:, :OWN].T
    return h2


# revision 8
# speedup vs baseline: 10.4090x; 1.2496x over previous
"""GraphSAGE 2-layer encoder on 8 TRN2 NeuronCores — single-launch design.

Strategy (dst-sharded, on-device gather, one launch for both layers):
- Nodes sharded 8x12500 by dst range; core k computes output rows for its
  nodes. x ships bf16-sharded (3.2MB/core); an on-device AllGather builds the
  full node table in DRAM.
- Per layer, per core: per-edge messages are gathered ON DEVICE from the full
  table via indirect DMA (128 rows per instruction, one per 128-slot tile);
  segment-sum runs on the PE as one-hot matmuls (S built by iota-compare on
  DVE), accumulating feature-major aggregates in PSUM banks; DVE scales by
  1/deg; stationary W_l/W_r matmuls transform; the self term h_own^T comes
  from PE-transposing own rows loaded from the local shard; ScalarE fuses
  bias+ReLU.
- Layer 1's output is PE-transposed back to node-major bf16, stored to the
  local shard, and AllGathered into a second full table for layer 2's
  gathers. Layer 2 writes the transposed output shard [128, N_CANON] bf16.
- One compiled program, one launch: host only shards inputs and reassembles
  the output.
"""
import os
import numpy as np
import ml_dtypes

import concourse.bass as bass
import concourse.tile as tile
from concourse import bacc, mybir
from concourse.masks import make_identity

N_NODES = 100000
N_CORES = 8
OWN = N_NODES // N_CORES          # 12500
D = 128
CELL = 128                        # node-columns per cell (= S width = MM N)
N_CELLS = (OWN + CELL - 1) // CELL  # 98
N_CANON = N_CELLS * CELL          # 12544
BANK_CELLS = 4                    # cells per PSUM bank (512 cols)
N_BANKS = (N_CELLS + BANK_CELLS - 1) // BANK_CELLS  # 25
FULL_CELLS = OWN // CELL          # 97 full cells
TAIL_ROWS = OWN - FULL_CELLS * CELL  # 84 rows in the last (partial) cell

BF16 = mybir.dt.bfloat16
F32 = mybir.dt.float32
F32R = mybir.dt.float32r
I32 = mybir.dt.int32

_cache = {}


def _build_program(T_cells):
    """Both layers' SPMD program. T_cells[c] = #128-slot tiles for cell c."""
    TOT_T = int(np.sum(T_cells))
    nc = bacc.Bacc(num_devices=N_CORES)

    xsh_d = nc.declare_dram_parameter("xsh", [OWN, D], BF16, isOutput=False)
    idx_d = nc.declare_dram_parameter("idx", [128, max(TOT_T, 1)], I32, isOutput=False)
    dstc_d = nc.declare_dram_parameter("dstc", [128, max(TOT_T, 1)], BF16, isOutput=False)
    inv_d = nc.declare_dram_parameter("invc", [1, N_CANON], F32, isOutput=False)
    wl0_d = nc.declare_dram_parameter("wl0", [128, 128], F32R, isOutput=False)
    wr0_d = nc.declare_dram_parameter("wr0", [128, 128], F32R, isOutput=False)
    wl1_d = nc.declare_dram_parameter("wl1", [128, 128], F32R, isOutput=False)
    wr1_d = nc.declare_dram_parameter("wr1", [128, 128], F32R, isOutput=False)
    b0_d = nc.declare_dram_parameter("b0", [128, 1], F32, isOutput=False)
    b1_d = nc.declare_dram_parameter("b1", [128, 1], F32, isOutput=False)
    iota_d = nc.declare_dram_parameter("iota", [1, CELL], BF16, isOutput=False)
    out_d = nc.declare_dram_parameter("outT", [128, N_CANON], BF16, isOutput=True)

    xb = nc.dram_tensor("xb", [OWN, D], BF16, kind="Internal")
    h_full = nc.dram_tensor("hfull", [N_NODES, D], BF16, kind="Internal")
    h1_sh = nc.dram_tensor("h1sh", [OWN, D], BF16, kind="Internal")
    h1_full = nc.dram_tensor("h1full", [N_NODES, D], BF16, kind="Internal")

    # bank plan: (cell_start, n_cells, tiles=[(t_global, cell_off_in_bank)])
    banks = []
    t0 = 0
    for bk in range(N_BANKS):
        c0 = bk * BANK_CELLS
        ncell = min(BANK_CELLS, N_CELLS - c0)
        tiles = []
        for ci in range(ncell):
            for _ in range(T_cells[c0 + ci]):
                tiles.append((t0, ci))
                t0 += 1
        banks.append((c0, ncell, tiles))
    T_BANK_MAX = max(max(len(b[2]) for b in banks), 1)

    with tile.TileContext(nc) as tc:
        with (
            tc.tile_pool(name="singles", bufs=1) as singles,
            tc.tile_pool(name="msgp", bufs=3) as msgp,
            tc.tile_pool(name="sp", bufs=3) as sp,
            tc.tile_pool(name="cellp", bufs=3) as cellp,
            tc.tile_pool(name="htp", bufs=2) as htp,
            tc.tile_pool(name="mp", bufs=2) as mp,
            tc.tile_pool(name="outp", bufs=3) as outp,
            tc.tile_pool(name="h1cp", bufs=3) as h1cp,
            tc.tile_pool(name="psa", bufs=3, space="PSUM") as psa,
            tc.tile_pool(name="pst", bufs=2, space="PSUM") as pst,
            tc.tile_pool(name="ptr", bufs=2, space="PSUM") as ptr,
        ):
            # ---- constants ----
            idx_t = singles.tile([128, max(TOT_T, 1)], I32)
            nc.sync.dma_start(out=idx_t[:], in_=idx_d[:])
            dstc_t = singles.tile([128, max(TOT_T, 1)], BF16)
            nc.sync.dma_start(out=dstc_t[:], in_=dstc_d[:])
            iota_t = singles.tile([128, CELL], BF16)
            nc.gpsimd.dma_start(
                out=iota_t[:],
                in_=bass.AP(tensor=iota_d[:].tensor, offset=0, ap=[[0, 128], [1, CELL]]),
            )
            inv_t = singles.tile([128, N_CANON], F32)
            nc.gpsimd.dma_start(
                out=inv_t[:],
                in_=bass.AP(tensor=inv_d[:].tensor, offset=0, ap=[[0, 128], [1, N_CANON]]),
            )
            wl0_t = singles.tile([128, 128], F32R)
            nc.sync.dma_start(out=wl0_t[:], in_=wl0_d[:])
            wr0_t = singles.tile([128, 128], F32R)
            nc.sync.dma_start(out=wr0_t[:], in_=wr0_d[:])
            wl1_t = singles.tile([128, 128], F32R)
            nc.sync.dma_start(out=wl1_t[:], in_=wl1_d[:])
            wr1_t = singles.tile([128, 128], F32R)
            nc.sync.dma_start(out=wr1_t[:], in_=wr1_d[:])
            b0_t = singles.tile([128, 1], F32)
            nc.sync.dma_start(out=b0_t[:], in_=b0_d[:])
            b1_t = singles.tile([128, 1], F32)
            nc.sync.dma_start(out=b1_t[:], in_=b1_d[:])
            zeros_t = singles.tile([128, 512], BF16)
            nc.vector.memset(zeros_t[:], 0.0)
            ident_b = singles.tile([128, 128], BF16)
            make_identity(nc, ident_b[:])
            ident_f = singles.tile([128, 128], F32)
            make_identity(nc, ident_f[:])

            # ---- AllGather x ----
            nc.gpsimd.dma_start(out=xb[:], in_=xsh_d[:])
            nc.gpsimd.collective_compute(
                "AllGather", mybir.AluOpType.bypass,
                replica_groups=[list(range(N_CORES))],
                ins=[xb[:]], outs=[h_full[:]],
            )

            def layer(src_full, self_src, wl_t, wr_t, b_t, is_last):
                for bk, (c0, ncell, tiles) in enumerate(banks):
                    bankcols = ncell * CELL
                    nt = len(tiles)
                    psum_agg = psa.tile([128, bankcols], F32)
                    nc.tensor.matmul(
                        psum_agg[:], zeros_t[:, :128], zeros_t[:, :bankcols],
                        start=True, stop=(nt == 0),
                    )
                    if nt:
                        tg0 = tiles[0][0]
                        msg_t = msgp.tile([128, T_BANK_MAX, D], BF16)
                        for i in range(nt):
                            nc.gpsimd.indirect_dma_start(
                                out=msg_t[:, i, :],
                                out_offset=None,
                                in_=src_full[:],
                                in_offset=bass.IndirectOffsetOnAxis(
                                    ap=idx_t[:, tg0 + i : tg0 + i + 1], axis=0
                                ),
                            )
                        s_t = sp.tile([128, T_BANK_MAX, CELL], BF16)
                        dap = dstc_t[:, tg0 : tg0 + nt].to_broadcast([128, nt, CELL])
                        iap = bass.AP(
                            tensor=iota_t[:].tensor, offset=iota_t[:].offset,
                            ap=[iota_t[:].ap[0], [0, nt], [1, CELL]],
                        )
                        nc.vector.tensor_tensor(
                            out=s_t[:, :nt, :], in0=dap, in1=iap,
                            op=mybir.AluOpType.is_equal,
                        )
                        for i, (tg, ci) in enumerate(tiles):
                            nc.tensor.matmul(
                                psum_agg[:, ci * CELL : (ci + 1) * CELL],
                                msg_t[:, i, :],
                                s_t[:, i, :],
                                start=False,
                                stop=(i == nt - 1),
                            )
                    # mean^T = psum * inv_cnt
                    mean_t = mp.tile([128, bankcols], F32R)
                    nc.vector.tensor_tensor(
                        out=mean_t[:], in0=psum_agg[:],
                        in1=inv_t[:, c0 * CELL : c0 * CELL + bankcols],
                        op=mybir.AluOpType.mult,
                    )
                    # self term: h_own^T per cell via PE transpose
                    ht_t = htp.tile([128, bankcols], F32R)
                    for ci in range(ncell):
                        cell = c0 + ci
                        r0 = cell * CELL
                        rows = min(CELL, OWN - r0)
                        if rows <= 0:
                            continue
                        cell_t = cellp.tile([128, 128], BF16)
                        nc.sync.dma_start(
                            out=cell_t[:rows, :], in_=self_src[r0 : r0 + rows, :]
                        )
                        psum_tr = ptr.tile([128, 128], BF16)
                        nc.tensor.transpose(psum_tr[:], cell_t[:], ident_b[:])
                        nc.vector.tensor_copy(
                            out=ht_t[:, ci * CELL : (ci + 1) * CELL], in_=psum_tr[:]
                        )
                    # transform: out^T = W_l^T mean^T + W_r^T h_own^T
                    psum_o = pst.tile([128, bankcols], F32)
                    nc.tensor.matmul(psum_o[:], wl_t[:], mean_t[:], start=True, stop=False)
                    nc.tensor.matmul(psum_o[:], wr_t[:], ht_t[:], start=False, stop=True)
                    out_t = outp.tile([128, bankcols], F32)
                    nc.scalar.activation(
                        out=out_t[:], in_=psum_o[:],
                        func=mybir.ActivationFunctionType.Relu,
                        bias=b_t[:], scale=1.0,
                    )
                    if is_last:
                        ob = h1cp.tile([128, bankcols], BF16)
                        nc.vector.tensor_copy(out=ob[:], in_=out_t[:])
                        nc.sync.dma_start(
                            out=out_d[:, c0 * CELL : c0 * CELL + bankcols], in_=ob[:]
                        )
                    else:
                        # node-major bf16 rows into h1_sh for next layer
                        for ci in range(ncell):
                            cell = c0 + ci
                            r0 = cell * CELL
                            rows = min(CELL, OWN - r0)
                            if rows <= 0:
                                continue
                            psum_tr = ptr.tile([128, 128], F32)
                            nc.tensor.transpose(
                                psum_tr[:],
                                out_t[:, ci * CELL : (ci + 1) * CELL],
                                ident_f[:],
                            )
                            h1c = h1cp.tile([128, 128], BF16)
                            nc.vector.tensor_copy(out=h1c[:], in_=psum_tr[:])
                            nc.sync.dma_start(
                                out=h1_sh[r0 : r0 + rows, :], in_=h1c[:rows, :]
                            )

            layer(h_full, xsh_d, wl0_t, wr0_t, b0_t, is_last=False)
            nc.gpsimd.collective_compute(
                "AllGather", mybir.AluOpType.bypass,
                replica_groups=[list(range(N_CORES))],
                ins=[h1_sh[:]], outs=[h1_full[:]],
            )
            layer(h1_full, h1_sh, wl1_t, wr1_t, b1_t, is_last=True)
    nc.finalize()
    return nc


def _schedule(edge_index):
    """Per-core slot schedule shared by both layers."""
    src = np.asarray(edge_index[0], dtype=np.int64)
    dst = np.asarray(edge_index[1], dtype=np.int64)
    deg = np.bincount(dst, minlength=N_NODES).astype(np.float32)
    inv_full = 1.0 / np.maximum(deg, 1.0)

    cores = []
    cell_counts = np.zeros((N_CORES, N_CELLS), np.int64)
    for k in range(N_CORES):
        m = (dst // OWN) == k
        s_k = src[m]
        dloc = dst[m] - k * OWN
        order = np.argsort(dloc, kind="stable")
        s_k, dloc = s_k[order], dloc[order]
        cell = dloc // CELL
        cell_counts[k] = np.bincount(cell, minlength=N_CELLS)
        cores.append((s_k, dloc, cell))

    T_cells = np.ceil(cell_counts.max(axis=0) / 128.0).astype(np.int64)
    TOT_T = int(T_cells.sum())
    TOT_S = TOT_T * 128
    tile_base = np.concatenate([[0], np.cumsum(T_cells)])[:-1]
    slot_base = tile_base * 128

    sched = []
    for k in range(N_CORES):
        s_k, dloc, cell = cores[k]
        n = len(s_k)
        cnt = cell_counts[k]
        cstart = np.concatenate([[0], np.cumsum(cnt)])[:-1]
        rank = np.arange(n) - cstart[cell]
        slot = slot_base[cell] + rank
        slot_src = np.zeros(TOT_S, np.int64)
        slot_src[slot] = s_k
        dstc_flat = np.full(TOT_S, -1.0, np.float32)
        dstc_flat[slot] = (dloc % CELL).astype(np.float32)
        # slot s -> (t = s//128, p = s%128); device reads [p, t]
        dstc_arr = dstc_flat.reshape(TOT_T, 128).T.astype(ml_dtypes.bfloat16)
        idx_arr = slot_src.reshape(TOT_T, 128).T.astype(np.int32)
        inv_row = np.ones((1, N_CANON), np.float32)
        inv_row[0, :OWN] = inv_full[k * OWN : (k + 1) * OWN]
        sched.append((
            np.ascontiguousarray(idx_arr),
            np.ascontiguousarray(dstc_arr),
            inv_row,
        ))
    return sched, T_cells, TOT_T


def _in_maps(sched, x, W_l0, b_l0, W_r0, W_l1, b_l1, W_r1):
    x_bf = x.astype(ml_dtypes.bfloat16)
    iota = np.arange(CELL).astype(ml_dtypes.bfloat16).reshape(1, CELL)
    f32 = lambda a: np.ascontiguousarray(a.astype(np.float32))
    maps = []
    for k in range(N_CORES):
        idx_arr, dstc_arr, inv_row = sched[k]
        maps.append({
            "xsh": np.ascontiguousarray(x_bf[k * OWN : (k + 1) * OWN]),
            "idx": idx_arr,
            "dstc": dstc_arr,
            "invc": inv_row,
            "wl0": f32(W_l0), "wr0": f32(W_r0),
            "wl1": f32(W_l1), "wr1": f32(W_r1),
            "b0": f32(b_l0).reshape(128, 1), "b1": f32(b_l1).reshape(128, 1),
            "iota": iota,
        })
    return maps


def _make_runner(nc):
    """SPMD PJRT runner mirroring bass2jax.run_bass_via_pjrt's multi-core
    path, with the jitted callable cached across calls and the donated
    output buffers created on-device (no host->device zero transfer)."""
    import jax
    import jax.numpy as jnp
    from jax.experimental.shard_map import shard_map
    from jax.sharding import Mesh, PartitionSpec, NamedSharding
    from concourse import bass2jax

    bass2jax.install_neuronx_cc_hook()
    assert nc.dbg_addr is None

    partition_name = nc.partition_id_tensor.name if nc.partition_id_tensor else None
    in_names, out_names, out_avals = [], [], []
    for alloc in nc.m.functions[0].allocations:
        if not isinstance(alloc, mybir.MemoryLocationSet):
            continue
        name = alloc.memorylocations[0].name
        if alloc.kind == "ExternalInput":
            if name != partition_name:
                in_names.append(name)
        elif alloc.kind == "ExternalOutput":
            out_names.append(name)
            out_avals.append(
                jax.core.ShapedArray(tuple(alloc.tensor_shape), mybir.dt.np(alloc.dtype))
            )
    n_params = len(in_names)
    n_outs = len(out_names)
    all_names = list(in_names) + list(out_names)
    if partition_name is not None:
        all_names.append(partition_name)
    donate = tuple(range(n_params, n_params + n_outs))

    def _body(*args):
        operands = list(args)
        if partition_name is not None:
            operands.append(bass2jax.partition_id_tensor())
        outs = bass2jax._bass_exec_p.bind(
            *operands,
            out_avals=tuple(out_avals),
            in_names=tuple(all_names),
            out_names=tuple(out_names),
            lowering_input_output_aliases=(),
            sim_require_finite=True,
            sim_require_nnan=True,
            nc=nc,
        )
        return tuple(outs)

    devices = jax.devices()[:N_CORES]
    assert len(devices) == N_CORES
    mesh = Mesh(np.asarray(devices), ("core",))
    in_specs = (PartitionSpec("core"),) * (n_params + n_outs)
    out_specs = (PartitionSpec("core"),) * n_outs
    sharded = jax.jit(
        shard_map(_body, mesh=mesh, in_specs=in_specs, out_specs=out_specs,
                  check_rep=False),
        donate_argnums=donate, keep_unused=True,
    )
    shardings = tuple(NamedSharding(mesh, PartitionSpec("core")) for _ in out_names)
    zero_fn = jax.jit(
        lambda: tuple(
            jnp.zeros((N_CORES * a.shape[0], *a.shape[1:]), a.dtype) for a in out_avals
        ),
        out_shardings=shardings,
    )

    def run(in_maps):
        per_core = [[np.asarray(m[name]) for name in in_names] for m in in_maps]
        concat_in = [
            np.concatenate([per_core[c][i] for c in range(N_CORES)], axis=0)
            for i in range(n_params)
        ]
        outs = sharded(*concat_in, *zero_fn())
        return {
            name: np.asarray(outs[i]).reshape(N_CORES, *out_avals[i].shape)
            for i, name in enumerate(out_names)
        }

    return run


def kernel(x, edge_index, W_l0, b_l0, W_r0, W_l1, b_l1, W_r1):
    import time as _time
    x = np.asarray(x, dtype=np.float32)

    sched, T_cells, TOT_T = _schedule(edge_index)
    tkey = tuple(T_cells.tolist())
    if tkey not in _cache:
        nc = _build_program(T_cells)
        _cache[tkey] = _make_runner(nc)
    run = _cache[tkey]

    maps = _in_maps(sched, x, W_l0, b_l0, W_r0, W_l1, b_l1, W_r1)

    def launch():
        t0 = _time.perf_counter()
        res = run(maps)
        wall = int((_time.perf_counter() - t0) * 1e9)
        return res, wall

    res, w1 = launch()          # includes NEFF compile on first call
    res, w2 = launch()          # warm launch: transfer + execute + fetch
    kernel.last_exec_ns = min(w1, w2)

    outT = res["outT"].astype(np.float32)  # [N_CORES, 128, N_CANON]
    h2 = np.empty((N_NODES, D), np.float32)
    for k in range(N_CORES):
        h2[k * OWN : (k + 1) * OWN] = outT[k][:, :OWN].T
    return h2
